# revision 2
# baseline (speedup 1.0000x reference)
"""Trainium2 Bass kernel for nn_Correlation (max_disp=4).

out[b, k, h, w] = mean_c x1[b,c,h,w] * pad(x2)[b,c,h+dx,w+dy],
k = 9*dx + dy, dx,dy in [0,9), pad = 4 zeros on each spatial side.

Strategy (per core, batch-parallel over 8 cores):
  All-pairs matmul over the channel contraction. For each output w-column
  w0 and each 32-row h-chunk c, one matmul computes every (h', dy)
  cross-correlation at once:

    psum[h, (h'-32c)*9 + dy] = sum_c x1[c,h,w0] * x2p[c,h',w0+dy]
      lhsT = x1[:, 32c:32c+32, w0]        [C, 32]   (M=32, tile_position 32c)
      rhs  = x2p[:, 32c:32c+40, w0:w0+9]  [C, 40, 9] (N=360)

  The 81 outputs for pixel (h, w0) sit at psum[h, 9*(h%32) + (9*dx+dy)] —
  one contiguous 81-run per partition. Four chunk-matmuls share one PSUM
  bank; a full-width Vector/Act copy casts the band to fp16 in SBUF with
  the 1/128 channel-mean folded in; contiguous DMAs ship it to DRAM and
  the host does the final fixed-stride gather.

  Two h-phases (chunks {0,1} then {2,3}) let the PE start after only half
  the inputs are resident; input loads are chunked and interleaved.
"""

import sys

if "/opt/trn_rl_repo" not in sys.path:
    sys.path.insert(0, "/opt/trn_rl_repo")

import numpy as np

B, C, H, W = 8, 128, 128, 128
D = 4
ND = 2 * D + 1  # 9
NK = ND * ND  # 81
PH, PW = H + 2 * D, W + 2 * D  # 136, 136
BAND = 40 * ND  # 360 columns per chunk-matmul
NPHASE = 2
CHUNKS = ((0, 1), (2, 3))  # h-chunks per phase
GROUP = 4  # w0 per psum tile
BLK = 16  # w0 per staging buffer / out-DMA

_cache = {}


def _build():
    from contextlib import ExitStack

    import concourse.mybir as mybir
    from concourse import bacc
    from concourse.bass import MemorySpace
    from concourse.tile import TileContext

    f32 = mybir.dt.float32
    f16 = mybir.dt.float16

    nc = bacc.Bacc("TRN2", target_bir_lowering=False, debug=False)
    X1 = nc.declare_dram_parameter("x1", [C, H, W], f32, isOutput=False)
    X2 = nc.declare_dram_parameter("x2", [C, H, W], f32, isOutput=False)
    # out_s[phase, blk, hrel, slot, col]: band rows for h = 64*phase + hrel,
    # w0 = 16*blk + slot.  Host gathers out[k,h,w0] = band[h,w0,9*(h%32)+k].
    OUTS = nc.declare_dram_parameter(
        "out_s", [NPHASE, W // BLK, H // NPHASE, BLK, BAND], f16, isOutput=True
    )

    with TileContext(nc) as tc, ExitStack() as ctx:
        consts = ctx.enter_context(tc.tile_pool(name="consts", bufs=1))
        psums = ctx.enter_context(
            tc.tile_pool(name="psums", bufs=2, space=MemorySpace.PSUM)
        )

        x1_sb = consts.tile([C, H, W], f16)
        x2tmp = consts.tile([C, H, W], f16)
        x2p = consts.tile([C, PH, PW], f16)
        s_sb = consts.tile([C, 2, BLK, BAND], f16)  # double-buffered staging

        # zero borders of the padded x2 (interior overwritten below)
        nc.vector.memset(x2p[:, 0:D, :], 0.0)
        nc.vector.memset(x2p[:, D + H :, :], 0.0)
        nc.vector.memset(x2p[:, D : D + H, 0:D], 0.0)
        nc.vector.memset(x2p[:, D : D + H, D + W :], 0.0)

        # interleaved chunked loads (SWDGE casts f32->fp16).  x2 chunks of
        # 36 rows so h-chunk c depends only on x2-chunks {c-1, c}.
        x2rows = [(0, 36), (36, 72), (72, 108), (108, 128)]
        for q in range(4):
            r0, r1 = x2rows[q]
            nc.gpsimd.dma_start(x2tmp[:, r0:r1, :], X2[:, r0:r1, :])
            nc.gpsimd.dma_start(
                x1_sb[:, 32 * q : 32 * q + 32, :], X1[:, 32 * q : 32 * q + 32, :]
            )
            nc.scalar.copy(
                x2p[:, D + r0 : D + r1, D : D + W], x2tmp[:, r0:r1, :]
            )

        ngrp = W // GROUP  # 32 psum groups per phase
        for phase in range(NPHASE):
            for g in range(ngrp):
                P = psums.tile([128, GROUP, 512], f32)
                for c in CHUNKS[phase]:
                    for s in range(GROUP):
                        w0 = GROUP * g + s
                        nc.tensor.matmul(
                            P[32 * c : 32 * c + 32, s, 0:BAND],
                            x1_sb[:, 32 * c : 32 * c + 32, w0],
                            x2p[:, 32 * c : 32 * c + 40, w0 : w0 + ND],
                            start=True,
                            stop=True,
                            tile_position=(0, 32 * c),
                        )
                # evacuate band to fp16 staging, folding in the 1/C mean
                p0 = 64 * phase
                buf = (g // 4) % 2
                sl = GROUP * (g % 4)
                src = P[p0 : p0 + 64, :, 0:BAND]
                dst = s_sb[p0 : p0 + 64, buf, sl : sl + GROUP, :]
                if g % 2 == 0:
                    nc.vector.tensor_scalar_mul(dst, src, 1.0 / C)
                else:
                    nc.scalar.mul(dst, src, 1.0 / C)
                if g % 4 == 3:  # 16 w0 staged -> ship to DRAM
                    blk = g // 4
                    nc.sync.dma_start(
                        OUTS[phase, blk],
                        s_sb[p0 : p0 + 64, buf],
                    )

    nc.finalize()
    return nc


def _get_program():
    if "prog" not in _cache:
        _cache["prog"] = _build()
    return _cache["prog"]


def _run(x_1, x_2, trace=False):
    from concourse.bass_utils import run_bass_kernel_spmd

    nc = _get_program()
    x_1 = np.ascontiguousarray(np.asarray(x_1, dtype=np.float32))
    x_2 = np.ascontiguousarray(np.asarray(x_2, dtype=np.float32))
    in_maps = [{"x1": x_1[i], "x2": x_2[i]} for i in range(B)]
    res = run_bass_kernel_spmd(nc, in_maps, core_ids=list(range(B)), trace=trace)

    # host-side gather: band[h, w0, 9*(h%32)+k] -> out[k, h, w0]
    idx = (9 * (np.arange(H) % 32))[:, None, None] + np.arange(NK)[None, None, :]
    out = np.empty((B, NK, H, W), dtype=np.float32)
    for i in range(B):
        o = res.results[i]["out_s"]  # [2, 8, 64, 16, 360] fp16
        band = o.transpose(0, 2, 1, 3, 4).reshape(H, W, BAND)
        g = np.take_along_axis(band, idx, axis=2)  # [H, W, NK]
        out[i] = g.transpose(2, 0, 1).astype(np.float32)
    return out, res


def kernel(x_1, x_2):
    out, _ = _run(x_1, x_2)
    return out


# revision 17
# speedup vs baseline: 2.0240x; 2.0240x over previous
"""Trainium2 Bass kernel for nn_Correlation (max_disp=4).

out[b, k, h, w] = mean_c x1[b,c,h,w] * pad(x2)[b,c,h+dx,w+dy],
k = 9*dx + dy, dx,dy in [0,9), pad = 4 zeros on each spatial side.

Strategy (per core, batch-parallel over 8 cores):
  All-pairs matmul over the channel contraction with 8x8 pixel blocks.
  For a block at (h0, w0), one matmul computes every (h', w') pairing:

    psum[8*hi+wi, 16*h'rel + w'rel]
        = sum_c x1[c, h0+hi, w0+wi] * x2p[c, h0+h'rel, w0+w'rel]
      lhsT = x1[:, h0:h0+8, w0:w0+8]     [C, 8, 8]   (M=64 per stacked pair)
      rhs  = x2p[:, h0:h0+16, w0:w0+16]  [C, 16, 16] (N=256)

  Output element (dx,dy) of pixel (hi,wi) sits at column (hi+dx)*16 +
  (wi+dy) — the 16x16 window covers the 9x9 displacements of all 64
  pixels exactly.  Two vertically adjacent blocks stack into the 128
  psum partitions (M=64 at tile_position 0 / 64); two stacks share one
  PSUM bank.  A full-width Vector/Act copy casts each bank to fp16 in
  SBUF with the 1/C channel-mean folded in; contiguous DMAs ship the
  band to DRAM and the host does the final fixed-stride gather.

  Loads are chunked and interleaved so the PE starts after ~20 rows of
  x2 and 16 rows of x1 are resident.
"""

import sys

if "/opt/trn_rl_repo" not in sys.path:
    sys.path.insert(0, "/opt/trn_rl_repo")

import numpy as np

B, C, H, W = 8, 128, 128, 128
D = 4
ND = 2 * D + 1  # 9
NK = ND * ND  # 81
PH, PW = H + 2 * D, W + 2 * D  # 136, 136
HB = WB = 8  # pixel block
NCOL = (HB + 2 * D) * (WB + 2 * D)  # 256 psum columns per block
NW0 = W // WB  # 16 w-blocks
NH0 = H // (2 * HB)  # 8 stacked h-groups (16 rows each)
NTILE = NH0 * NW0 // 2  # 64 psum bank-tiles (2 stacks each)
SBUF_N = 4  # staging ring buffers (2 bank-tiles each)

_cache = {}


def _build():
    from contextlib import ExitStack

    import concourse.mybir as mybir
    from concourse import bacc
    from concourse.bass import MemorySpace
    from concourse.tile import TileContext

    f32 = mybir.dt.float32
    f16 = mybir.dt.float16

    nc = bacc.Bacc("TRN2", target_bir_lowering=False, debug=False)
    X1 = nc.declare_dram_parameter("x1", [C, H, W], f32, isOutput=False)
    X2 = nc.declare_dram_parameter("x2", [C, H, W], f32, isOutput=False)
    # out_s[i, p, :]: bands of bank-tile pair i (2 tiles x 2 stacks x 256)
    OUTS = nc.declare_dram_parameter(
        "out_s", [NTILE // 2, C, 4 * NCOL], f16, isOutput=True
    )

    with TileContext(nc) as tc, ExitStack() as ctx:
        consts = ctx.enter_context(tc.tile_pool(name="consts", bufs=1))
        psums = ctx.enter_context(
            tc.tile_pool(name="psums", bufs=8, space=MemorySpace.PSUM)
        )

        x1_sb = consts.tile([C, H, W], f16)
        # blocked x1: [c, h-blk, w-blk, hi, wi] so a weight block is one
        # contiguous 64-elem run (matmul weights allow only 1 free dim)
        x1b = consts.tile([C, H // HB, NW0, HB, WB], f16)
        x2tmp = consts.tile([C, H, W], f16)
        x2p = consts.tile([C, PH, PW], f16)
        s_sb = consts.tile([C, SBUF_N, 4 * NCOL], f16)  # staging ring

        # zero borders of the padded x2 (interior overwritten below)
        nc.vector.memset(x2p[:, 0:D, :], 0.0)
        nc.vector.memset(x2p[:, D + H :, :], 0.0)
        nc.vector.memset(x2p[:, D : D + H, 0:D], 0.0)
        nc.vector.memset(x2p[:, D : D + H, D + W :], 0.0)

        # interleaved chunked loads (SWDGE casts f32->fp16).  x2 chunk
        # boundaries chosen so the h-group at H0 needs only chunks
        # <= H0/16 + 1; pad copies run on DVE (fp16 both sides, 2x mode).
        x2rows = [(0, 20)] + [(20 + 16 * k, min(36 + 16 * k, H)) for k in range(7)]
        for q in range(8):
            r0, r1 = x2rows[q]
            nc.gpsimd.dma_start(x2tmp[:, r0:r1, :], X2[:, r0:r1, :])
            nc.gpsimd.dma_start(
                x1_sb[:, 16 * q : 16 * q + 16, :], X1[:, 16 * q : 16 * q + 16, :]
            )
            nc.vector.tensor_scalar_mul(
                x2p[:, D + r0 : D + r1, D : D + W], x2tmp[:, r0:r1, :], 1.0
            )
            # rearrange the fresh x1 chunk into blocked layout
            for hb in (2 * q, 2 * q + 1):
                src = x1_sb[:, HB * hb : HB * hb + HB, :].rearrange(
                    "c hi (wb wi) -> c wb hi wi", wb=NW0
                )
                nc.scalar.copy(x1b[:, hb], src)

        # stacks ordered h-major for load pipelining
        tile_i = 0
        P = None
        for hg in range(NH0):  # 16-row h-group
            for wb in range(NW0):
                st = tile_i % 2
                if st == 0:
                    P = psums.tile([128, 2, NCOL], f32)
                for sub in range(2):  # vertical 8-row sub-blocks
                    h0 = 16 * hg + 8 * sub
                    w0 = WB * wb
                    nc.tensor.matmul(
                        P[64 * sub : 64 * sub + 64, st, :],
                        x1b[:, h0 // HB, wb],
                        x2p[:, h0 : h0 + HB + 2 * D, w0 : w0 + WB + 2 * D],
                        start=True,
                        stop=True,
                        tile_position=(0, 64 * sub),
                    )
                if st == 1:
                    # evacuate 2 stacks to fp16 staging with the 1/C mean
                    t = tile_i // 2  # bank-tile index
                    buf = (t // 2) % SBUF_N
                    half = (t % 2) * 2 * NCOL
                    src = P[:, :, :]
                    dst = s_sb[:, buf, half : half + 2 * NCOL]
                    if t % 2 == 0:
                        nc.vector.tensor_scalar_mul(dst, src, 1.0 / C)
                    else:
                        nc.scalar.mul(dst, src, 1.0 / C)
                    if t % 2 == 1:  # 2 bank-tiles staged -> ship
                        nc.sync.dma_start(OUTS[t // 2], s_sb[:, buf, :])
                tile_i += 1

    nc.finalize()
    return nc


def _get_program():
    if "prog" not in _cache:
        _cache["prog"] = _build()
    return _cache["prog"]


# host gather index: for partition p (sub, hi, wi) and k=(dx,dy):
#   col = (hi+dx)*16 + (wi+dy)
def _gather_idx():
    p = np.arange(C)
    hi = (p % 64) // 8
    wi = p % 8
    k = np.arange(NK)
    dx, dy = k // ND, k % ND
    return ((hi[:, None] + dx[None, :]) * 16 + wi[:, None] + dy[None, :]).astype(
        np.intp
    )  # [128, 81]


def _run(x_1, x_2, trace=False):
    from concourse.bass_utils import run_bass_kernel_spmd

    nc = _get_program()
    x_1 = np.ascontiguousarray(np.asarray(x_1, dtype=np.float32))
    x_2 = np.ascontiguousarray(np.asarray(x_2, dtype=np.float32))
    in_maps = [{"x1": x_1[i], "x2": x_2[i]} for i in range(B)]
    res = run_bass_kernel_spmd(nc, in_maps, core_ids=list(range(B)), trace=trace)

    idx = _gather_idx()[None, :, :]  # [1, 128, 81]
    out = np.empty((B, NK, H, W), dtype=np.float32)
    for i in range(B):
        o = res.results[i]["out_s"]  # [32, 128, 1024] fp16
        # [pair, p, (tile, stack, col)] -> stacks in original order
        band = o.reshape(NTILE // 2, C, 4, NCOL).transpose(0, 2, 1, 3)
        band = band.reshape(NTILE * 2, C, NCOL)  # [stack, p, 256]
        g = np.take_along_axis(band, idx, axis=2)  # [stack, p, 81]
        # stack = (hg, wb); p = (sub, hi, wi); out[k, h, w]
        g = g.reshape(NH0, NW0, 2, HB, WB, NK)
        out[i] = (
            g.transpose(5, 0, 2, 3, 1, 4).reshape(NK, H, W).astype(np.float32)
        )
    return out, res


def kernel(x_1, x_2):
    out, _ = _run(x_1, x_2)
    return out


# revision 20
# speedup vs baseline: 2.3490x; 1.1606x over previous
"""Trainium2 Bass kernel for nn_Correlation (max_disp=4).

out[b, k, h, w] = mean_c x1[b,c,h,w] * pad(x2)[b,c,h+dx,w+dy],
k = 9*dx + dy, dx,dy in [0,9), pad = 4 zeros on each spatial side.

Strategy (per core, batch-parallel over 8 cores):
  All-pairs matmul over the channel contraction with 8x8 pixel blocks.
  For a block at (h0, w0), one matmul computes every (h', w') pairing:

    psum[8*hi+wi, 16*h'rel + w'rel]
        = sum_c x1[c, h0+hi, w0+wi] * x2p[c, h0+h'rel, w0+w'rel]
      lhsT = x1[:, h0:h0+8, w0:w0+8]     [C, 8, 8]   (M=64 per stacked pair)
      rhs  = x2p[:, h0:h0+16, w0:w0+16]  [C, 16, 16] (N=256)

  Output element (dx,dy) of pixel (hi,wi) sits at column (hi+dx)*16 +
  (wi+dy) — the 16x16 window covers the 9x9 displacements of all 64
  pixels exactly.  Two vertically adjacent blocks stack into the 128
  psum partitions (M=64 at tile_position 0 / 64); two stacks share one
  PSUM bank.  A full-width Vector/Act copy casts each bank to fp16 in
  SBUF with the 1/C channel-mean folded in; contiguous DMAs ship the
  band to DRAM and the host does the final fixed-stride gather.

  Loads are chunked and interleaved so the PE starts after ~20 rows of
  x2 and 16 rows of x1 are resident.
"""

import sys

if "/opt/trn_rl_repo" not in sys.path:
    sys.path.insert(0, "/opt/trn_rl_repo")

import numpy as np

B, C, H, W = 8, 128, 128, 128
D = 4
ND = 2 * D + 1  # 9
NK = ND * ND  # 81
PH, PW = H + 2 * D, W + 2 * D  # 136, 136
HB = WB = 8  # pixel block
NCOL = (HB + 2 * D) * (WB + 2 * D)  # 256 psum columns per block
NW0 = W // WB  # 16 w-blocks
NH0 = H // (2 * HB)  # 8 stacked h-groups (16 rows each)
NTILE = NH0 * NW0 // 2  # 64 psum bank-tiles (2 stacks each)
SBUF_N = 8  # staging ring buffers (2 bank-tiles each)

_cache = {}


def _build():
    from contextlib import ExitStack

    import concourse.mybir as mybir
    from concourse import bacc
    from concourse.bass import MemorySpace
    from concourse.tile import TileContext

    f32 = mybir.dt.float32
    f16 = mybir.dt.float16

    nc = bacc.Bacc("TRN2", target_bir_lowering=False, debug=False)
    X1 = nc.declare_dram_parameter("x1", [C, H, W], f32, isOutput=False)
    X2 = nc.declare_dram_parameter("x2", [C, H, W], f32, isOutput=False)
    # out_s[i, p, :]: bands of bank-tile pair i (2 tiles x 2 stacks x 256)
    OUTS = nc.declare_dram_parameter(
        "out_s", [NTILE // 2, C, 4 * NCOL], f16, isOutput=True
    )

    with TileContext(nc) as tc, ExitStack() as ctx:
        consts = ctx.enter_context(tc.tile_pool(name="consts", bufs=1))
        psums = ctx.enter_context(
            tc.tile_pool(name="psums", bufs=8, space=MemorySpace.PSUM)
        )

        x1_sb = consts.tile([C, H, W], f16)
        # blocked x1: [c, h-blk, w-blk, hi, wi] so a weight block is one
        # contiguous 64-elem run (matmul weights allow only 1 free dim)
        x1b = consts.tile([C, H // HB, NW0, HB, WB], f16)
        x2tmp = consts.tile([C, H, W], f16)
        x2p = consts.tile([C, PH, PW], f16)
        s_sb = consts.tile([C, SBUF_N, 4 * NCOL], f16)  # staging ring

        # zero borders of the padded x2 (interior overwritten below)
        nc.vector.memset(x2p[:, 0:D, :], 0.0)
        nc.vector.memset(x2p[:, D + H :, :], 0.0)
        nc.vector.memset(x2p[:, D : D + H, 0:D], 0.0)
        nc.vector.memset(x2p[:, D : D + H, D + W :], 0.0)

        # interleaved chunked loads (SWDGE casts f32->fp16).  x2 chunk
        # boundaries chosen so the h-group at H0 needs only chunks
        # <= H0/16 + 1; pad copies run on DVE (fp16 both sides, 2x mode).
        x2rows = [(0, 20)] + [(20 + 16 * k, min(36 + 16 * k, H)) for k in range(7)]
        for q in range(8):
            r0, r1 = x2rows[q]
            nc.gpsimd.dma_start(
                x1_sb[:, 16 * q : 16 * q + 16, :], X1[:, 16 * q : 16 * q + 16, :]
            )
            nc.gpsimd.dma_start(x2tmp[:, r0:r1, :], X2[:, r0:r1, :])
            # rearrange the fresh x1 chunk into blocked layout (DVE 2x)
            for hb in (2 * q, 2 * q + 1):
                src = x1_sb[:, HB * hb : HB * hb + HB, :].rearrange(
                    "c hi (wb wi) -> c wb hi wi", wb=NW0
                )
                nc.vector.tensor_scalar_mul(x1b[:, hb], src, 1.0)
            nc.vector.tensor_scalar_mul(
                x2p[:, D + r0 : D + r1, D : D + W], x2tmp[:, r0:r1, :], 1.0
            )

        # stacks ordered h-major for load pipelining
        tile_i = 0
        P = None
        for hg in range(NH0):  # 16-row h-group
            for wb in range(NW0):
                st = tile_i % 2
                if st == 0:
                    P = psums.tile([128, 2, NCOL], f32)
                for sub in range(2):  # vertical 8-row sub-blocks
                    h0 = 16 * hg + 8 * sub
                    w0 = WB * wb
                    nc.tensor.matmul(
                        P[64 * sub : 64 * sub + 64, st, :],
                        x1b[:, h0 // HB, wb],
                        x2p[:, h0 : h0 + HB + 2 * D, w0 : w0 + WB + 2 * D],
                        start=True,
                        stop=True,
                        tile_position=(0, 64 * sub),
                    )
                if st == 1:
                    # evacuate 2 stacks to fp16 staging with the 1/C mean
                    t = tile_i // 2  # bank-tile index
                    buf = (t // 2) % SBUF_N
                    half = (t % 2) * 2 * NCOL
                    src = P[:, :, :]
                    dst = s_sb[:, buf, half : half + 2 * NCOL]
                    if t % 4 == 3:  # DVE also carries loads/rearranges
                        nc.vector.tensor_scalar_mul(dst, src, 1.0 / C)
                    else:
                        nc.scalar.mul(dst, src, 1.0 / C)
                    if t % 2 == 1:  # 2 bank-tiles staged -> ship
                        nc.sync.dma_start(OUTS[t // 2], s_sb[:, buf, :])
                tile_i += 1

    nc.finalize()
    return nc


def _get_program():
    if "prog" not in _cache:
        _cache["prog"] = _build()
    return _cache["prog"]


# host gather index: for partition p (sub, hi, wi) and k=(dx,dy):
#   col = (hi+dx)*16 + (wi+dy)
def _gather_idx():
    p = np.arange(C)
    hi = (p % 64) // 8
    wi = p % 8
    k = np.arange(NK)
    dx, dy = k // ND, k % ND
    return ((hi[:, None] + dx[None, :]) * 16 + wi[:, None] + dy[None, :]).astype(
        np.intp
    )  # [128, 81]


def _run(x_1, x_2, trace=False):
    from concourse.bass_utils import run_bass_kernel_spmd

    nc = _get_program()
    x_1 = np.ascontiguousarray(np.asarray(x_1, dtype=np.float32))
    x_2 = np.ascontiguousarray(np.asarray(x_2, dtype=np.float32))
    in_maps = [{"x1": x_1[i], "x2": x_2[i]} for i in range(B)]
    res = run_bass_kernel_spmd(nc, in_maps, core_ids=list(range(B)), trace=trace)

    idx = _gather_idx()[None, :, :]  # [1, 128, 81]
    out = np.empty((B, NK, H, W), dtype=np.float32)
    for i in range(B):
        o = res.results[i]["out_s"]  # [32, 128, 1024] fp16
        # [pair, p, (tile, stack, col)] -> stacks in original order
        band = o.reshape(NTILE // 2, C, 4, NCOL).transpose(0, 2, 1, 3)
        band = band.reshape(NTILE * 2, C, NCOL)  # [stack, p, 256]
        g = np.take_along_axis(band, idx, axis=2)  # [stack, p, 81]
        # stack = (hg, wb); p = (sub, hi, wi); out[k, h, w]
        g = g.reshape(NH0, NW0, 2, HB, WB, NK)
        out[i] = (
            g.transpose(5, 0, 2, 3, 1, 4).reshape(NK, H, W).astype(np.float32)
        )
    return out, res


def kernel(x_1, x_2):
    out, _ = _run(x_1, x_2)
    return out
